# revision 5
# baseline (speedup 1.0000x reference)
"""Trainium2 Bass kernel: batched chamfer-style metric (nn_Metric_56985626083917).

Reference computation per batch b (B=8, N=M=4096, D=3):
    sqd[n,m] = |pred_n - gt_m|^2   (clamped >= 0)
    dist1 = sqrt(min_m sqd)  [N] ; dist2 = sqrt(min_n sqd)  [M]
    loss_b = mean(dist1)+mean(dist2) + 3*(mean(top2048(dist1))+mean(top2048(dist2)))
    out = mean_b loss_b

Strategy: data-parallel, one batch per NeuronCore (8 cores).
Per core the kernel computes zt[n,m] = -sqd[n,m] via a single K=16 fp16
matmul with error-compensated hi/lo splits (fp32-grade accuracy at full
bf16/fp16 PE rate):
    zt = sum_c 2*p_c*g_c - |p|^2 - |g|^2
rows: per coord c 4 slots (2ph*gh, 2ph*gl, 2pl*gh, 2pl*gl), plus 2 slots for
-|p|^2 (hi/lo vs ones) and 2 slots for -|g|^2.

Main loop per 128-row tile of pred x 2048-col chunk of gt:
    PE   : 4 matmuls -> PSUM [128,2048] fp32
    ACT  : copy/convert PSUM -> SBUF fp16 (enables DVE 2x mode)
    DVE  : tensor_tensor max  -> running column-max   (dist2 path)
           tensor_tensor_reduce max -> per-row max chain (dist1 path)
Tail: PE transposes of running colmax + DVE reduce -> dist2 layout [128,32];
relu(-x); ACT sqrt; device binary-search (30 iters) for the top-2048
threshold of each dist array; sums folded across partitions with a
ones-matmul. Host combines 8 scalars per core into the final loss.
"""

import os
import sys

import numpy as np

for _p in ("/opt/trn_rl_repo",):
    if os.path.isdir(_p) and _p not in sys.path:
        sys.path.insert(0, _p)

import concourse.bass as bass  # noqa: E402
import concourse.mybir as mybir  # noqa: E402
import concourse.tile as tile  # noqa: E402
from concourse import bacc  # noqa: E402
from concourse.bass_utils import run_bass_kernel_spmd  # noqa: E402
from concourse.masks import make_identity  # noqa: E402

B = 8
N = 4096  # pred points per batch
M = 4096  # gt points per batch
P = 128  # partitions
KSLOTS = 16
NTILE = N // P  # 32
MCHUNK = 2048
NCHUNK = M // MCHUNK  # 2
K1 = N // 2  # top-k count (PERCENT=0.5)
WEIGHT = 3.0
BS_ITERS = 30
BS_HI = 16.0  # distances are < 16 for these inputs (|p|,|g| ~ N(0,1))

F16 = mybir.dt.float16
F32 = mybir.dt.float32
Alu = mybir.AluOpType

TRACE = False
LAST_RESULT = None

_CACHE = {}


def _build_nc(reps=1):
    nc = bacc.Bacc(
        "TRN2", target_bir_lowering=False, debug=False, num_devices=B
    )
    a_in = nc.dram_tensor("A", [KSLOTS, N], F16, kind="ExternalInput")
    g_in = nc.dram_tensor("G", [KSLOTS, M], F16, kind="ExternalInput")
    out8 = nc.dram_tensor("OUT8", [1, 8], F32, kind="ExternalOutput")
    d12 = nc.dram_tensor("D12", [P, 2 * NTILE], F32, kind="ExternalOutput")

    with tile.TileContext(nc) as tc:
        for _ in range(reps):
            _body(nc, tc, a_in, g_in, out8, d12)
    nc.compile()
    return nc


def _body(nc, tc, a_in, g_in, out8, d12):
    from contextlib import ExitStack

    with ExitStack() as ctx:
        const = ctx.enter_context(tc.tile_pool(name="const", bufs=1))
        io = ctx.enter_context(tc.tile_pool(name="io", bufs=1))
        wpool = ctx.enter_context(tc.tile_pool(name="w", bufs=4))
        junkp = ctx.enter_context(tc.tile_pool(name="junk", bufs=2))
        runp = ctx.enter_context(tc.tile_pool(name="run", bufs=1))
        smallp = ctx.enter_context(tc.tile_pool(name="small", bufs=1))

        A = io.tile([KSLOTS, N], F16)
        G = io.tile([KSLOTS, M], F16)
        nc.sync.dma_start(out=A, in_=a_in[:])
        nc.sync.dma_start(out=G, in_=g_in[:])

        run2 = runp.tile([P, M], F16)
        nc.vector.memset(run2, -60000.0)
        R1 = smallp.tile([P, NTILE], F32, name="R1")
        Rpart = smallp.tile([P, NTILE * NCHUNK], F32, name="Rpart")

        # ---------- main loop ----------
        with tc.tile_pool(name="ps_main", bufs=2, space="PSUM") as psum:
            for i in range(NTILE):
                for jj in range(NCHUNK):
                    ps = psum.tile([P, MCHUNK], F32)
                    for k in range(MCHUNK // 512):
                        nc.tensor.matmul(
                            ps[:, k * 512 : (k + 1) * 512],
                            A[:, i * P : (i + 1) * P],
                            G[:, jj * MCHUNK + k * 512 : jj * MCHUNK + (k + 1) * 512],
                            start=True,
                            stop=True,
                        )
                    w = wpool.tile([P, MCHUNK], F16, tag="w")
                    nc.scalar.copy(w, ps)
                    # colmax accumulate (dist2 path)
                    nc.vector.tensor_tensor(
                        run2[:, jj * MCHUNK : (jj + 1) * MCHUNK],
                        w,
                        run2[:, jj * MCHUNK : (jj + 1) * MCHUNK],
                        op=Alu.max,
                    )
                    # rowmax partials (dist1 path): accum_out = max-reduce
                    junk = junkp.tile([P, MCHUNK], F16, tag="junk")
                    s = i * NCHUNK + jj
                    nc.vector.tensor_scalar(
                        out=junk,
                        in0=w,
                        scalar1=-1.0e30,
                        scalar2=None,
                        op0=Alu.max,
                        op1=Alu.max,
                        accum_out=Rpart[:, s : s + 1],
                    )

        # ---------- tail ----------
        # combine rowmax partials: [P, NTILE, NCHUNK] -> max over NCHUNK
        nc.vector.tensor_reduce(
            out=R1,
            in_=Rpart.rearrange("p (i j) -> p i j", j=NCHUNK),
            axis=mybir.AxisListType.X,
            op=Alu.max,
        )
        ident = const.tile([P, P], F16, name="ident")
        make_identity(nc, ident)
        ones_f32 = const.tile([P, P], F32, name="ones_f32")
        nc.vector.memset(ones_f32, 1.0)

        D2R = smallp.tile([P, NTILE], F32, name="D2R")
        with tc.tile_pool(name="ps_t", bufs=4, space="PSUM") as psum_t:
            for k in range(NTILE):
                ps_t = psum_t.tile([P, P], F16, tag="pst")
                nc.tensor.transpose(ps_t, run2[:, k * P : (k + 1) * P], ident)
                nc.vector.tensor_reduce(
                    out=D2R[:, k : k + 1], in_=ps_t, axis=mybir.AxisListType.X,
                    op=Alu.max,
                )

        # sqd = relu(-x); dist = sqrt(sqd)
        Dall = smallp.tile([P, 2 * NTILE], F32, name="Dall")
        Sall = smallp.tile([P, 2 * NTILE], F32, name="Sall")
        nc.vector.tensor_scalar(
            out=Sall[:, 0:NTILE], in0=R1, scalar1=-1.0, scalar2=0.0,
            op0=Alu.mult, op1=Alu.max,
        )
        nc.vector.tensor_scalar(
            out=Sall[:, NTILE : 2 * NTILE], in0=D2R, scalar1=-1.0, scalar2=0.0,
            op0=Alu.mult, op1=Alu.max,
        )
        nc.scalar.sqrt(Dall, Sall)
        nc.sync.dma_start(out=d12[:], in_=Dall)

        # ---------- device binary search for top-K1 thresholds ----------
        Tsb = smallp.tile([P, 2], F32, name="Tsb")
        nc.vector.memset(Tsb, BS_HI / 2.0)
        cnt_sb = smallp.tile([P, 2], F32, name="cnt_sb")
        bjunk = smallp.tile([P, 2 * NTILE], F32, name="bjunk")
        g2 = smallp.tile([P, 2], F32, name="g2")

        with tc.tile_pool(name="ps_bs", bufs=2, space="PSUM") as psum_bs:
            delta = BS_HI / 4.0
            for it in range(BS_ITERS):
                for a in range(2):
                    nc.vector.tensor_scalar(
                        out=bjunk[:, a * NTILE : (a + 1) * NTILE],
                        in0=Dall[:, a * NTILE : (a + 1) * NTILE],
                        scalar1=Tsb[:, a : a + 1],
                        scalar2=None,
                        op0=Alu.is_gt,
                        op1=Alu.add,
                        accum_out=cnt_sb[:, a : a + 1],
                    )
                c_ps = psum_bs.tile([P, 2], F32, tag="cps")
                nc.tensor.matmul(c_ps, ones_f32, cnt_sb, start=True, stop=True)
                nc.vector.tensor_scalar(
                    out=g2, in0=c_ps, scalar1=float(K1) - 0.5,
                    scalar2=2.0 * delta, op0=Alu.is_gt, op1=Alu.mult,
                )
                nc.vector.scalar_tensor_tensor(
                    out=Tsb, in0=g2, scalar=delta, in1=Tsb,
                    op0=Alu.subtract, op1=Alu.add,
                )
                delta *= 0.5

            # ---------- final sums ----------
            Fin = smallp.tile([P, 8], F32, name="Fin")
            for a in range(2):
                sl = slice(a * NTILE, (a + 1) * NTILE)
                nc.vector.tensor_reduce(
                    out=Fin[:, a : a + 1], in_=Dall[:, sl],
                    axis=mybir.AxisListType.X, op=Alu.add,
                )
                nc.vector.scalar_tensor_tensor(
                    out=bjunk[:, sl], in0=Dall[:, sl],
                    scalar=Tsb[:, a : a + 1], in1=Dall[:, sl],
                    op0=Alu.is_gt, op1=Alu.mult,
                    accum_out=Fin[:, 2 + a : 3 + a],
                )
                nc.vector.tensor_scalar(
                    out=bjunk[:, sl], in0=Dall[:, sl],
                    scalar1=Tsb[:, a : a + 1], scalar2=None,
                    op0=Alu.is_gt, op1=Alu.add,
                    accum_out=Fin[:, 4 + a : 5 + a],
                )
            nc.vector.tensor_copy(Fin[:, 6:8], Tsb)

            f_ps = psum_bs.tile([1, 8], F32, tag="fps")
            nc.tensor.matmul(f_ps, ones_f32[:, 0:1], Fin, start=True, stop=True)
            F8 = smallp.tile([1, 8], F32, name="F8")
            nc.scalar.copy(F8, f_ps)
            nc.sync.dma_start(out=out8[:], in_=F8)


def _split16(x):
    hi = x.astype(np.float16)
    lo = (x - hi.astype(np.float64)).astype(np.float16)
    return hi, lo


def _prep(pred, gt):
    """Build the [16, 4096] fp16 stationary/moving operand matrices."""
    p = pred.astype(np.float64)
    g = gt.astype(np.float64)
    ph, pl = _split16(p)  # [N,3] each
    gh, gl = _split16(g)
    pt = ph.astype(np.float64) + pl.astype(np.float64)
    gt_ = gh.astype(np.float64) + gl.astype(np.float64)
    pn = (pt * pt).sum(-1)  # [N]
    gn = (gt_ * gt_).sum(-1)  # [M]
    pnh, pnl = _split16(-pn)
    gnh, gnl = _split16(-gn)

    A = np.zeros((KSLOTS, N), np.float16)
    G = np.zeros((KSLOTS, M), np.float16)
    for c in range(3):
        r = 4 * c
        A[r + 0] = 2.0 * ph[:, c]
        A[r + 1] = 2.0 * ph[:, c]
        A[r + 2] = 2.0 * pl[:, c]
        A[r + 3] = 2.0 * pl[:, c]
        G[r + 0] = gh[:, c]
        G[r + 1] = gl[:, c]
        G[r + 2] = gh[:, c]
        G[r + 3] = gl[:, c]
    A[12] = pnh
    A[13] = pnl
    G[12] = 1.0
    G[13] = 1.0
    A[14] = 1.0
    A[15] = 1.0
    G[14] = gnh
    G[15] = gnl
    return A, G


def _get_nc():
    if "nc" not in _CACHE:
        _CACHE["nc"] = _build_nc()
    return _CACHE["nc"]


def kernel(pred_pc, gt_pc):
    global LAST_RESULT
    pred_pc = np.asarray(pred_pc)
    gt_pc = np.asarray(gt_pc)
    nc = _get_nc()
    in_maps = []
    for b in range(B):
        A, G = _prep(pred_pc[b], gt_pc[b])
        in_maps.append({"A": A, "G": G})
    res = run_bass_kernel_spmd(nc, in_maps, list(range(B)), trace=TRACE)
    LAST_RESULT = res
    losses = []
    for b in range(B):
        o = np.asarray(res.results[b]["OUT8"], np.float64).reshape(-1)
        s1, s2, st1, st2, c1, c2, t1s, t2s = o
        t1 = t1s / P
        t2 = t2s / P
        top1 = st1 + (K1 - c1) * t1
        top2 = st2 + (K1 - c2) * t2
        loss_cd = s1 / N + s2 / M
        loss_w = top1 / K1 + top2 / K1
        losses.append(loss_cd + WEIGHT * loss_w)
    return np.array(np.mean(losses), dtype=np.float32)


# revision 17
# speedup vs baseline: 2.5994x; 2.5994x over previous
"""Trainium2 Bass kernel: batched chamfer-style metric (nn_Metric_56985626083917).

Reference computation per batch b (B=8, N=M=4096, D=3):
    sqd[n,m] = |pred_n - gt_m|^2   (clamped >= 0)
    dist1 = sqrt(min_m sqd)  [N] ; dist2 = sqrt(min_n sqd)  [M]
    loss_b = mean(dist1)+mean(dist2) + 3*(mean(top2048(dist1))+mean(top2048(dist2)))
    out = mean_b loss_b

Strategy: data-parallel, one batch per NeuronCore (8 cores).
Per core the device computes zt[n,m] = -sqd[n,m] via a single K=16 fp16
matmul with error-compensated hi/lo splits (fp32-grade accuracy at full
fp16 PE rate):
    zt = sum_c 2*p_c*g_c - |p|^2 - |g|^2
slots: per coord c 4 products (ph*gh, ph*gl, pl*gh, pl*gl), plus 2 slots for
-|p|^2 (hi/lo vs ones) and 2 slots for -|g|^2.

Main loop per 128-row pred tile (32 iterations):
    PE   : matmuls -> PSUM [128, MCHUNK] fp32 (zt tile)
    ACT  : copy/convert PSUM -> SBUF fp16
    DVE  : tensor_tensor max      -> running column-max  (dist2 path)
           tensor_scalar max-accum -> per-row max        (dist1 path)
Tail: 7 partition-halving tensor_tensor max folds collapse the column-max
to one row. Device outputs row maxes [128, NTILE*NCHUNK] and column maxes
[1, 4096]; the host (O(N) work) does relu/sqrt, means, and exact top-k
via np.partition, then averages the 8 per-batch losses.
"""

import os
import sys

import numpy as np

for _p in ("/opt/trn_rl_repo",):
    if os.path.isdir(_p) and _p not in sys.path:
        sys.path.insert(0, _p)

import concourse.bass as bass  # noqa: E402
import concourse.mybir as mybir  # noqa: E402
import concourse.tile as tile  # noqa: E402
from concourse import bacc  # noqa: E402
from concourse.bass_utils import run_bass_kernel_spmd  # noqa: E402

B = 8
N = 4096  # pred points per batch
M = 4096  # gt points per batch
P = 128  # partitions
KSLOTS = 13
NTILE = N // P  # 32
MCHUNK = int(os.environ.get("KERN_MCHUNK", "4096"))
NCHUNK = M // MCHUNK
MM_N = int(os.environ.get("KERN_MM_N", "512"))  # moving free dim per matmul (<= 1 PSUM bank)
K1 = N // 2  # top-k count (PERCENT=0.5)
WEIGHT = 3.0

F16 = mybir.dt.float16
F32 = mybir.dt.float32
Alu = mybir.AluOpType

LAST_RESULT = None
_CACHE = {}


def _build_nc(reps=1):
    nc = bacc.Bacc(
        "TRN2", target_bir_lowering=False, debug=False, num_devices=B
    )
    a_in = nc.dram_tensor("A", [KSLOTS, N], F16, kind="ExternalInput")
    g_in = nc.dram_tensor("G", [KSLOTS, M], F16, kind="ExternalInput")
    r1_out = nc.dram_tensor("R1OUT", [P, NTILE * NCHUNK], F32, kind="ExternalOutput")
    c2_out = nc.dram_tensor("C2OUT", [32, M], F16, kind="ExternalOutput")

    with tile.TileContext(nc) as tc:
        for _ in range(reps):
            _body(nc, tc, a_in, g_in, r1_out, c2_out)
    nc.compile()
    return nc


def _body(nc, tc, a_in, g_in, r1_out, c2_out):
    from contextlib import ExitStack

    with ExitStack() as ctx:
        io = ctx.enter_context(tc.tile_pool(name="io", bufs=1))
        wpool = ctx.enter_context(tc.tile_pool(name="w", bufs=2))
        runp = ctx.enter_context(tc.tile_pool(name="run", bufs=1))
        smallp = ctx.enter_context(tc.tile_pool(name="small", bufs=1))

        A = io.tile([KSLOTS, N], F16)
        G = io.tile([KSLOTS, M], F16)
        nc.sync.dma_start(out=A, in_=a_in[:])
        nc.sync.dma_start(out=G, in_=g_in[:])

        run2 = runp.tile([P, M], F16)
        nc.vector.memset(run2, -60000.0)
        Rpart = smallp.tile([P, NTILE * NCHUNK], F32, name="Rpart")

        # ---------- main loop ----------
        ps_bufs = 2 if MCHUNK <= 2048 else 1
        with tc.tile_pool(name="ps_main", bufs=ps_bufs, space="PSUM") as psum:
            for i in range(NTILE):
                for jj in range(NCHUNK):
                    ps = psum.tile([P, MCHUNK], F32)
                    for kk in range(MCHUNK // MM_N):
                        nc.tensor.matmul(
                            ps[:, kk * MM_N : (kk + 1) * MM_N],
                            A[:, i * P : (i + 1) * P],
                            G[:, jj * MCHUNK + kk * MM_N : jj * MCHUNK + (kk + 1) * MM_N],
                            start=True,
                            stop=True,
                        )
                    w = wpool.tile([P, MCHUNK], F16, tag="w")
                    nc.scalar.copy(w, ps)
                    # colmax accumulate (dist2 path)
                    nc.vector.tensor_tensor(
                        run2[:, jj * MCHUNK : (jj + 1) * MCHUNK],
                        w,
                        run2[:, jj * MCHUNK : (jj + 1) * MCHUNK],
                        op=Alu.max,
                    )
                    # rowmax partial (dist1 path)
                    s = i * NCHUNK + jj
                    nc.vector.tensor_reduce(
                        out=Rpart[:, s : s + 1],
                        in_=w,
                        axis=mybir.AxisListType.X,
                        op=Alu.max,
                    )

        # ---------- tail: fold run2 across partitions (128 -> 32) ----------
        # TensorTensor requires equal base partitions for both SBUF inputs,
        # so DMA-realign the upper half to partition 0 before each fold.
        # Host finishes the 32 -> 1 fold.
        tmp = runp.tile([64, M], F16, name="ftmp")
        nc.sync.dma_start(out=tmp, in_=run2[64:128, :])
        nc.vector.tensor_tensor(run2[0:64, :], run2[0:64, :], tmp, op=Alu.max)
        nc.sync.dma_start(out=tmp[0:32, :], in_=run2[32:64, :])
        nc.vector.tensor_tensor(
            run2[0:32, :], run2[0:32, :], tmp[0:32, :], op=Alu.max
        )

        nc.sync.dma_start(out=r1_out[:], in_=Rpart)
        nc.sync.dma_start(out=c2_out[:], in_=run2[0:32, :])


def _split16(x):
    hi = x.astype(np.float16)
    lo = (x - hi.astype(np.float64)).astype(np.float16)
    return hi, lo


def _prep(pred, gt):
    """Build the [16, 4096] fp16 stationary/moving operand matrices."""
    p = pred.astype(np.float64)
    g = gt.astype(np.float64)
    ph, pl = _split16(p)  # [N,3] each
    gh, gl = _split16(g)
    pt = ph.astype(np.float64) + pl.astype(np.float64)
    gt_ = gh.astype(np.float64) + gl.astype(np.float64)
    pn = (pt * pt).sum(-1)  # [N]
    gn = (gt_ * gt_).sum(-1)  # [M]
    pnh, pnl = _split16(-pn)
    gnh, gnl = _split16(-gn)

    A = np.zeros((KSLOTS, N), np.float16)
    G = np.zeros((KSLOTS, M), np.float16)
    for c in range(3):
        r = 3 * c
        # (ph+pl)*(gh+gl) ~= ph*gh + ph*gl + pl*gh  (pl*gl ~ 2^-22, dropped)
        A[r + 0] = 2.0 * ph[:, c]
        A[r + 1] = 2.0 * ph[:, c]
        A[r + 2] = 2.0 * pl[:, c]
        G[r + 0] = gh[:, c]
        G[r + 1] = gl[:, c]
        G[r + 2] = gh[:, c]
    A[9] = pnh
    A[10] = pnl
    G[9] = 1.0
    G[10] = 1.0
    A[11] = 1.0
    A[12] = 1.0
    G[11] = gnh
    G[12] = gnl
    return A, G


def _get_nc():
    if "nc" not in _CACHE:
        _CACHE["nc"] = _build_nc()
    return _CACHE["nc"]


def _loss_from_dists(d):
    """d: [4096] distances. Returns mean + WEIGHT * mean(top half)."""
    k = K1
    part = np.partition(d, d.size - k)
    topk = part[d.size - k :]
    return d.mean() + 0.0, topk  # mean handled by caller


def kernel(pred_pc, gt_pc):
    global LAST_RESULT
    pred_pc = np.asarray(pred_pc)
    gt_pc = np.asarray(gt_pc)
    nc = _get_nc()
    in_maps = []
    for b in range(B):
        A, G = _prep(pred_pc[b], gt_pc[b])
        in_maps.append({"A": A, "G": G})
    res = run_bass_kernel_spmd(nc, in_maps, list(range(B)))
    LAST_RESULT = res
    losses = []
    for b in range(B):
        r1 = np.asarray(res.results[b]["R1OUT"], np.float32)
        c2 = np.asarray(res.results[b]["C2OUT"], np.float32).max(axis=0)  # [4096]
        # rowmax: combine NCHUNK partials per point
        r1 = r1.reshape(P, NTILE, NCHUNK).max(axis=2)  # [P, NTILE]
        d1 = np.sqrt(np.maximum(-r1, 0.0)).reshape(-1)  # [4096]
        d2 = np.sqrt(np.maximum(-c2, 0.0))  # [4096]
        loss = 0.0
        for d in (d1, d2):
            topk = np.partition(d, d.size - K1)[d.size - K1 :]
            loss += d.mean() + WEIGHT * topk.mean()
        losses.append(loss)
    return np.array(np.mean(losses), dtype=np.float32)


# revision 25
# speedup vs baseline: 3.0635x; 1.1786x over previous
"""Trainium2 Bass kernel: batched chamfer-style metric (nn_Metric_56985626083917).

Reference computation per batch b (B=8, N=M=4096, D=3):
    sqd[n,m] = |pred_n - gt_m|^2   (clamped >= 0)
    dist1 = sqrt(min_m sqd)  [N] ; dist2 = sqrt(min_n sqd)  [M]
    loss_b = mean(dist1)+mean(dist2) + 3*(mean(top2048(dist1))+mean(top2048(dist2)))
    out = mean_b loss_b

Strategy: data-parallel, one batch per NeuronCore (8 cores).
Per core the device computes zt[n,m] = -sqd[n,m] via a single K=16 fp16
matmul with error-compensated hi/lo splits (fp32-grade accuracy at full
fp16 PE rate):
    zt = sum_c 2*p_c*g_c - |p|^2 - |g|^2
slots: per coord c 4 products (ph*gh, ph*gl, pl*gh, pl*gl), plus 2 slots for
-|p|^2 (hi/lo vs ones) and 2 slots for -|g|^2.

slots: per coord c 3 products (ph*gh, ph*gl, pl*gh; pl*gl ~ 2^-22 dropped),
plus 2 slots for -|p|^2 (hi/lo vs ones) and 2 slots for -|g|^2 -> K=13.

Main loop per 128-row pred tile (32 iterations):
    PE   : 8 matmuls -> PSUM [128, 4096] fp32 (zt tile)
    DVE  : tensor_tensor max (PSUM, SBUF) -> running column-max (dist2 path)
           tensor_reduce max (PSUM)       -> per-row max        (dist1 path)
Tail: two partition folds (128 -> 32) with DMA realignment (the BIR
verifier requires equal base partitions for two-SBUF-input TensorTensor).
Device outputs row maxes [128, 32] and partially-folded column maxes
[32, 4096]; the host (O(N) work) finishes the fold, does relu/sqrt,
means, and exact top-k via np.partition, then averages the 8 losses.
"""

import os
import sys

import numpy as np

for _p in ("/opt/trn_rl_repo",):
    if os.path.isdir(_p) and _p not in sys.path:
        sys.path.insert(0, _p)

import concourse.bass as bass  # noqa: E402
import concourse.mybir as mybir  # noqa: E402
import concourse.tile as tile  # noqa: E402
from concourse import bacc  # noqa: E402
from concourse.bass_utils import run_bass_kernel_spmd  # noqa: E402

B = 8
N = 4096  # pred points per batch
M = 4096  # gt points per batch
P = 128  # partitions
KSLOTS = 13
NTILE = N // P  # 32
MCHUNK = 4096  # gt columns processed per pred tile iteration
NCHUNK = M // MCHUNK
MM_N = 512  # moving free dim per matmul (<= 1 PSUM bank)
K1 = N // 2  # top-k count (PERCENT=0.5)
WEIGHT = 3.0

F16 = mybir.dt.float16
F32 = mybir.dt.float32
Alu = mybir.AluOpType

LAST_RESULT = None
_CACHE = {}


def _build_nc(reps=1):
    nc = bacc.Bacc(
        "TRN2", target_bir_lowering=False, debug=False, num_devices=B
    )
    a_in = nc.dram_tensor("A", [KSLOTS, N], F16, kind="ExternalInput")
    g_in = nc.dram_tensor("G", [KSLOTS, M], F16, kind="ExternalInput")
    r1_out = nc.dram_tensor("R1OUT", [P, NTILE * NCHUNK], F32, kind="ExternalOutput")
    c2_out = nc.dram_tensor("C2OUT", [32, M], F32, kind="ExternalOutput")

    with tile.TileContext(nc) as tc:
        for _ in range(reps):
            _body(nc, tc, a_in, g_in, r1_out, c2_out)
    nc.compile()
    return nc


def _body(nc, tc, a_in, g_in, r1_out, c2_out):
    from contextlib import ExitStack

    with ExitStack() as ctx:
        io = ctx.enter_context(tc.tile_pool(name="io", bufs=1))
        runp = ctx.enter_context(tc.tile_pool(name="run", bufs=1))
        smallp = ctx.enter_context(tc.tile_pool(name="small", bufs=1))

        A = io.tile([KSLOTS, N], F16)
        G = io.tile([KSLOTS, M], F16)
        nc.sync.dma_start(out=A, in_=a_in[:])
        nc.sync.dma_start(out=G, in_=g_in[:])

        run2 = runp.tile([P, M], F32)
        nc.vector.memset(run2, -3.0e38)
        Rpart = smallp.tile([P, NTILE * NCHUNK], F32, name="Rpart")

        # ---------- main loop ----------
        ps_bufs = 2 if MCHUNK <= 2048 else 1
        with tc.tile_pool(name="ps_main", bufs=ps_bufs, space="PSUM") as psum:
            for i in range(NTILE):
                for jj in range(NCHUNK):
                    ps = psum.tile([P, MCHUNK], F32)
                    for kk in range(MCHUNK // MM_N):
                        nc.tensor.matmul(
                            ps[:, kk * MM_N : (kk + 1) * MM_N],
                            A[:, i * P : (i + 1) * P],
                            G[:, jj * MCHUNK + kk * MM_N : jj * MCHUNK + (kk + 1) * MM_N],
                            start=True,
                            stop=True,
                        )
                    # colmax accumulate (dist2 path) — read PSUM directly
                    nc.vector.tensor_tensor(
                        run2[:, jj * MCHUNK : (jj + 1) * MCHUNK],
                        ps,
                        run2[:, jj * MCHUNK : (jj + 1) * MCHUNK],
                        op=Alu.max,
                    )
                    # rowmax partial (dist1 path)
                    s = i * NCHUNK + jj
                    nc.vector.tensor_reduce(
                        out=Rpart[:, s : s + 1],
                        in_=ps,
                        axis=mybir.AxisListType.X,
                        op=Alu.max,
                    )

        # ---------- tail: fold run2 across partitions (128 -> 32) ----------
        # TensorTensor requires equal base partitions for both SBUF inputs,
        # so DMA-realign the upper half to partition 0 before each fold.
        # Host finishes the 32 -> 1 fold.
        tmp = runp.tile([64, M], F32, name="ftmp")
        nc.sync.dma_start(out=tmp, in_=run2[64:128, :])
        nc.vector.tensor_tensor(run2[0:64, :], run2[0:64, :], tmp, op=Alu.max)
        nc.sync.dma_start(out=tmp[0:32, :], in_=run2[32:64, :])
        nc.vector.tensor_tensor(
            run2[0:32, :], run2[0:32, :], tmp[0:32, :], op=Alu.max
        )

        nc.sync.dma_start(out=r1_out[:], in_=Rpart)
        nc.sync.dma_start(out=c2_out[:], in_=run2[0:32, :])


def _split16(x):
    hi = x.astype(np.float16)
    lo = (x - hi.astype(np.float64)).astype(np.float16)
    return hi, lo


def _prep(pred, gt):
    """Build the [16, 4096] fp16 stationary/moving operand matrices."""
    p = pred.astype(np.float64)
    g = gt.astype(np.float64)
    ph, pl = _split16(p)  # [N,3] each
    gh, gl = _split16(g)
    pt = ph.astype(np.float64) + pl.astype(np.float64)
    gt_ = gh.astype(np.float64) + gl.astype(np.float64)
    pn = (pt * pt).sum(-1)  # [N]
    gn = (gt_ * gt_).sum(-1)  # [M]
    pnh, pnl = _split16(-pn)
    gnh, gnl = _split16(-gn)

    A = np.zeros((KSLOTS, N), np.float16)
    G = np.zeros((KSLOTS, M), np.float16)
    for c in range(3):
        r = 3 * c
        # (ph+pl)*(gh+gl) ~= ph*gh + ph*gl + pl*gh  (pl*gl ~ 2^-22, dropped)
        A[r + 0] = 2.0 * ph[:, c]
        A[r + 1] = 2.0 * ph[:, c]
        A[r + 2] = 2.0 * pl[:, c]
        G[r + 0] = gh[:, c]
        G[r + 1] = gl[:, c]
        G[r + 2] = gh[:, c]
    A[9] = pnh
    A[10] = pnl
    G[9] = 1.0
    G[10] = 1.0
    A[11] = 1.0
    A[12] = 1.0
    G[11] = gnh
    G[12] = gnl
    return A, G


def _get_nc():
    if "nc" not in _CACHE:
        _CACHE["nc"] = _build_nc()
    return _CACHE["nc"]


def kernel(pred_pc, gt_pc):
    global LAST_RESULT
    pred_pc = np.asarray(pred_pc)
    gt_pc = np.asarray(gt_pc)
    nc = _get_nc()
    in_maps = []
    for b in range(B):
        A, G = _prep(pred_pc[b], gt_pc[b])
        in_maps.append({"A": A, "G": G})
    res = run_bass_kernel_spmd(nc, in_maps, list(range(B)))
    LAST_RESULT = res
    losses = []
    for b in range(B):
        r1 = np.asarray(res.results[b]["R1OUT"], np.float32)
        c2 = np.asarray(res.results[b]["C2OUT"], np.float32).max(axis=0)  # [4096]
        # rowmax: combine NCHUNK partials per point
        r1 = r1.reshape(P, NTILE, NCHUNK).max(axis=2)  # [P, NTILE]
        d1 = np.sqrt(np.maximum(-r1, 0.0)).reshape(-1)  # [4096]
        d2 = np.sqrt(np.maximum(-c2, 0.0))  # [4096]
        loss = 0.0
        for d in (d1, d2):
            topk = np.partition(d, d.size - K1)[d.size - K1 :]
            loss += d.mean() + WEIGHT * topk.mean()
        losses.append(loss)
    return np.array(np.mean(losses), dtype=np.float32)
